# revision 6
# baseline (speedup 1.0000x reference)
"""Trainium2 Bass kernel for nn_AttentionMHA: 8-way tensor-parallel over heads.

Full attention prefill: B=1, S=2048, D=4096, H=32 Q-heads, KVH=8 KV-heads,
HD=128, causal (input_pos = arange(S)).

Per-core sharding (core c of 8): Q heads 4c..4c+3, KV head c, wo columns
512c..512(c+1).  Pipeline per core:
  1. QKV projection (fp32r matmuls, contraction over D on partitions)
     producing q^T/k^T/v^T in [hd, token] layout.
  2. RoPE (half-split layout: DVE stream_shuffle + mul/add with host-built
     cos/sin tiles) + RMSNorm (GPSIMD partition reduce + PE outer-product
     broadcast of 1/rms).
  3. Causal attention per head: scores^T = k^T.T-style matmul into
     [key, query] tiles, ACT exp (scaled 1/sqrt(HD)), GPSIMD partition sums
     for the softmax denominator, PE accumulate y^T = v.T @ expS^T.
  4. AllGather of y^T shards -> full Y^T [4096, 2048].
  5. Output projection against the core's wo column shard -> out[:, 512c:].
Host concatenates the 8 column shards.
"""
import sys

sys.path.insert(0, "/opt/trn_rl_repo")

import numpy as np

import concourse.bass as bass
import concourse.tile as tile
from concourse import bacc, mybir

f32 = mybir.dt.float32
f32r = mybir.dt.float32r
AF = mybir.ActivationFunctionType
ALU = mybir.AluOpType

B, S, D = 1, 2048, 4096
H, KVH, HD = 32, 8, 128
NH = 4            # q heads per core
TB = 512          # token block
NT = S // TB      # 4 token blocks
KC = D // 128     # 32 contraction chunks
NKT = S // 128    # 16 key chunks
EPS = 1e-5
SCALE = 1.0 / np.sqrt(HD)
NEG = -30000.0
N_CORES = 8

SWAP_MASK = list(range(16, 32)) + list(range(0, 16))


def build_nc():
    nc = bacc.Bacc("TRN2", target_bir_lowering=False, debug=False,
                   num_devices=N_CORES)

    XT = nc.dram_tensor("XT", [D, S], f32, kind="ExternalInput")
    WQ = nc.dram_tensor("WQ", [D, NH * HD], f32, kind="ExternalInput")
    WK = nc.dram_tensor("WK", [D, HD], f32, kind="ExternalInput")
    WV = nc.dram_tensor("WV", [D, HD], f32, kind="ExternalInput")
    WO = nc.dram_tensor("WO", [D, 512], f32, kind="ExternalInput")
    CC = nc.dram_tensor("CC", [HD, S], f32, kind="ExternalInput")
    SS = nc.dram_tensor("SS", [HD, S], f32, kind="ExternalInput")
    MASK = nc.dram_tensor("MASK", [128, 4 * TB], f32, kind="ExternalInput")
    IDM = nc.dram_tensor("IDM", [128, 128], f32, kind="ExternalInput")
    WQKR = nc.dram_tensor("WQKR", [1, 128], f32, kind="ExternalInput")
    ONESR = nc.dram_tensor("ONESR", [1, 128], f32, kind="ExternalInput")
    ONESC = nc.dram_tensor("ONESC", [128, 1], f32, kind="ExternalInput")
    OUT = nc.dram_tensor("OUT", [S, 512], f32, kind="ExternalOutput")

    with tile.TileContext(nc) as tc:
        from contextlib import ExitStack
        with tc.tile_pool(name="dram", bufs=1, space="DRAM") as dram:
          y_ag = dram.tile([NH * HD, S], f32)
          y_full = dram.tile([H * HD, S], f32, addr_space="Shared")
          ctx = ExitStack()
          with ctx:
            const = ctx.enter_context(tc.tile_pool(name="const", bufs=1))
            wqpool_cm = ctx.enter_context(tc.tile_pool(name="wqpool", bufs=1))
            xtp = ctx.enter_context(tc.tile_pool(name="xtp", bufs=4))
            wkvp = ctx.enter_context(tc.tile_pool(name="wkvp", bufs=4))
            qfp = ctx.enter_context(tc.tile_pool(name="qfp", bufs=8))
            resid = ctx.enter_context(tc.tile_pool(name="resid", bufs=1))
            tmp = ctx.enter_context(tc.tile_pool(name="tmp", bufs=2))
            smalls = ctx.enter_context(tc.tile_pool(name="smalls", bufs=1))
            expp = ctx.enter_context(tc.tile_pool(name="expp", bufs=3))
            ystp = ctx.enter_context(tc.tile_pool(name="ystp", bufs=2))
            # PSUM: proj(3) + scores(2) + ypsum(1) + bcast(1) + denom(1) = 8 banks
            proj = ctx.enter_context(tc.tile_pool(name="proj", bufs=1, space="PSUM"))
            scoresp = ctx.enter_context(tc.tile_pool(name="scoresp", bufs=2, space="PSUM"))
            ypp = ctx.enter_context(tc.tile_pool(name="ypp", bufs=1, space="PSUM"))
            bcp = ctx.enter_context(tc.tile_pool(name="bcp", bufs=1, space="PSUM"))
            dpp = ctx.enter_context(tc.tile_pool(name="dpp", bufs=1, space="PSUM"))

            # ---- constants ----
            cc_t = const.tile([HD, S], f32)
            nc.sync.dma_start(cc_t[:], CC.ap())
            ss_t = const.tile([HD, S], f32)
            nc.sync.dma_start(ss_t[:], SS.ap())
            mask_t = const.tile([128, 4 * TB], f32)
            nc.sync.dma_start(mask_t[:], MASK.ap())
            id_t = const.tile([128, 128], f32)
            nc.sync.dma_start(id_t[:], IDM.ap())
            wqk_t = const.tile([1, 128], f32r)
            nc.sync.dma_start(wqk_t[:], WQKR.ap().bitcast(f32r))
            onesr_t = const.tile([1, 128], f32r)
            nc.sync.dma_start(onesr_t[:], ONESR.ap().bitcast(f32r))
            onesc_t = const.tile([128, 1], f32r)
            nc.sync.dma_start(onesc_t[:], ONESC.ap().bitcast(f32r))
            eps_t = const.tile([1, 1], f32)
            nc.vector.memset(eps_t[:], EPS)

            # wq resident: [128, KC, 512]
            wq_t = wqpool_cm.tile([128, KC, NH * HD], f32r)
            nc.sync.dma_start(
                wq_t[:], WQ.ap().bitcast(f32r).rearrange("(k p) n -> p k n", p=128))

            # persistent k/v
            kfin = resid.tile([128, S], f32r)         # k_final^T
            vnat = resid.tile([128, NKT * 128], f32r)  # v chunks [kt,128] packed

            xt_src = XT.ap().bitcast(f32r).rearrange("(k p) t -> k p t", p=128)
            wk_src = WK.ap().bitcast(f32r).rearrange("(k p) n -> k p n", p=128)
            wv_src = WV.ap().bitcast(f32r).rearrange("(k p) n -> k p n", p=128)

            def process_qk(raw_psum, is_k, j, head):
                """RoPE + RMSNorm from raw projection psum [128, TB].

                Writes q_final tile (returned) or k_final[:, j*TB:] for k.
                """
                qs = tmp.tile([128, TB], f32r, tag="qs")
                nc.scalar.copy(qs[:], raw_psum[:])
                sq = tmp.tile([128, TB], f32r, tag="sq")
                nc.scalar.square(sq[:], raw_psum[:])
                rsp = bcp.tile([1, TB], f32, tag="bc")
                nc.tensor.matmul(rsp[:], onesc_t[:], sq[:], start=True, stop=True)
                srt = smalls.tile([1, TB], f32, tag="srt")
                nc.scalar.activation(srt[:], rsp[:], AF.Sqrt,
                                     bias=eps_t[:], scale=1.0 / HD)
                rec = smalls.tile([1, TB], f32, tag="rec")
                nc.vector.reciprocal(rec[:], srt[:])
                recr = smalls.tile([1, TB], f32r, tag="recr")
                nc.vector.tensor_copy(recr[:], rec[:])
                bc = bcp.tile([128, TB], f32, tag="bc")
                lhs = wqk_t if is_k else onesr_t
                nc.tensor.matmul(bc[:], lhs[:], recr[:], start=True, stop=True)
                tsw = tmp.tile([128, TB], f32, tag="tsw")
                nc.vector.stream_shuffle(tsw[:], qs[:].bitcast(f32), SWAP_MASK)
                t1 = tmp.tile([128, TB], f32, tag="t1")
                nc.vector.tensor_tensor(
                    t1[:], qs[:].bitcast(f32), cc_t[:, j * TB:(j + 1) * TB], ALU.mult)
                t2 = tmp.tile([128, TB], f32, tag="t2")
                nc.vector.tensor_tensor(
                    t2[:], tsw[:], ss_t[:, j * TB:(j + 1) * TB], ALU.mult)
                nc.vector.tensor_tensor(t1[:], t1[:], t2[:], ALU.add)
                if is_k:
                    nc.vector.tensor_tensor(
                        kfin[:, j * TB:(j + 1) * TB], t1[:], bc[:], ALU.mult)
                    return None
                qf = qfp.tile([128, TB], f32r, tag="qf")
                nc.vector.tensor_tensor(qf[:], t1[:], bc[:], ALU.mult)
                return qf

            for j in range(NT):
                tok = slice(j * TB, (j + 1) * TB)
                # ---- QKV pass 1: q0, q1, k ----
                pq0 = proj.tile([128, TB], f32, tag="pa")
                pq1 = proj.tile([128, TB], f32, tag="pb")
                pk = proj.tile([128, TB], f32, tag="pc")
                for k in range(KC):
                    xt = xtp.tile([128, TB], f32r, tag="xt")
                    nc.sync.dma_start(xt[:], xt_src[k][:, tok])
                    wkc = wkvp.tile([128, HD], f32r, tag="wk")
                    nc.sync.dma_start(wkc[:], wk_src[k])
                    st, sp = (k == 0), (k == KC - 1)
                    nc.tensor.matmul(pq0[:], wq_t[:, k, 0:128], xt[:],
                                     start=st, stop=sp)
                    nc.tensor.matmul(pq1[:], wq_t[:, k, 128:256], xt[:],
                                     start=st, stop=sp)
                    nc.tensor.matmul(pk[:], wkc[:], xt[:], start=st, stop=sp)
                kf = process_qk(pk, True, j, None)
                q_tiles = [process_qk(pq0, False, j, 0),
                           process_qk(pq1, False, j, 1)]
                # ---- QKV pass 2: q2, q3, v ----
                pq2 = proj.tile([128, TB], f32, tag="pa")
                pq3 = proj.tile([128, TB], f32, tag="pb")
                pv = proj.tile([128, TB], f32, tag="pc")
                for k in range(KC):
                    xt = xtp.tile([128, TB], f32r, tag="xt")
                    nc.sync.dma_start(xt[:], xt_src[k][:, tok])
                    wvc = wkvp.tile([128, HD], f32r, tag="wv")
                    nc.sync.dma_start(wvc[:], wv_src[k])
                    st, sp = (k == 0), (k == KC - 1)
                    nc.tensor.matmul(pq2[:], wq_t[:, k, 256:384], xt[:],
                                     start=st, stop=sp)
                    nc.tensor.matmul(pq3[:], wq_t[:, k, 384:512], xt[:],
                                     start=st, stop=sp)
                    nc.tensor.matmul(pv[:], wvc[:], xt[:], start=st, stop=sp)
                q_tiles.append(process_qk(pq2, False, j, 2))
                q_tiles.append(process_qk(pq3, False, j, 3))
                # ---- v: transpose [hd, tok] -> natural [tok, hd] chunks ----
                vt_s = tmp.tile([128, TB], f32, tag="vts")
                nc.scalar.copy(vt_s[:], pv[:])
                for ci in range(4):
                    pt = bcp.tile([128, 128], f32, tag="bc")
                    nc.tensor.transpose(pt[:], vt_s[:, ci * 128:(ci + 1) * 128],
                                        id_t[:])
                    nc.vector.tensor_copy(
                        vnat[:, (4 * j + ci) * 128:(4 * j + ci + 1) * 128], pt[:])

                # ---- attention for token block j ----
                nchunks = 4 * (j + 1)
                for h in range(NH):
                    qf = q_tiles[h]
                    yp = ypp.tile([128, TB], f32, tag="yp")
                    dps = dpp.tile([1, TB], f32, tag="dp")
                    for c in range(nchunks):
                        sc = scoresp.tile([128, TB], f32, tag="sc")
                        nc.tensor.matmul(
                            sc[:], kfin[:, c * 128:(c + 1) * 128], qf[:],
                            start=True, stop=True)
                        if c >= 4 * j:  # diagonal chunk: causal mask
                            ci = c - 4 * j
                            nc.vector.tensor_tensor(
                                sc[:], sc[:], mask_t[:, ci * TB:(ci + 1) * TB],
                                ALU.add)
                        ex = expp.tile([128, TB], f32r, tag="ex")
                        nc.scalar.activation(ex[:], sc[:], AF.Exp, scale=SCALE)
                        nc.tensor.matmul(
                            dps[:], onesc_t[:], ex[:],
                            start=(c == 0), stop=(c == nchunks - 1))
                        nc.tensor.matmul(
                            yp[:], vnat[:, c * 128:(c + 1) * 128], ex[:],
                            start=(c == 0), stop=(c == nchunks - 1))
                    # normalize
                    drec = smalls.tile([1, TB], f32, tag="drec")
                    nc.vector.reciprocal(drec[:], dps[:])
                    drecr = smalls.tile([1, TB], f32r, tag="drecr")
                    nc.vector.tensor_copy(drecr[:], drec[:])
                    bcr = bcp.tile([128, TB], f32, tag="bc")
                    nc.tensor.matmul(bcr[:], onesr_t[:], drecr[:],
                                     start=True, stop=True)
                    bcs = tmp.tile([128, TB], f32, tag="bcs")
                    nc.scalar.copy(bcs[:], bcr[:])
                    yst = ystp.tile([128, TB], f32, tag="yst")
                    nc.vector.tensor_tensor(yst[:], yp[:], bcs[:], ALU.mult)
                    nc.sync.dma_start(y_ag[h * HD:(h + 1) * HD, tok], yst[:])

            # ---- AllGather y^T shards ----
            nc.gpsimd.collective_compute(
                "AllGather", ALU.bypass,
                replica_groups=[list(range(N_CORES))],
                ins=[y_ag[:]], outs=[y_full[:]])

          # ---- output projection (phase-1/2 pools released above) ----
          with tc.tile_pool(name="wop", bufs=1) as wop, \
               tc.tile_pool(name="ytp", bufs=2) as ytp, \
               tc.tile_pool(name="outp", bufs=4) as outp, \
               tc.tile_pool(name="pop", bufs=2, space="PSUM") as pop:
            wo_t = wop.tile([128, KC, 512], f32r)
            nc.sync.dma_start(
                wo_t[:],
                WO.ap().bitcast(f32r).rearrange("(k p) n -> p k n", p=128))
            yt_src = y_full[:].bitcast(f32r).rearrange(
                "(k p) t -> p k t", p=128)
            for t in range(S // 128):
                yt = ytp.tile([128, KC, 128], f32r, tag="yt")
                nc.sync.dma_start(yt[:], yt_src[:, :, t * 128:(t + 1) * 128])
                po = pop.tile([128, 512], f32, tag="po")
                for k in range(KC):
                    nc.tensor.matmul(po[:], yt[:, k, :], wo_t[:, k, :],
                                     start=(k == 0), stop=(k == KC - 1))
                ot = outp.tile([128, 512], f32, tag="ot")
                nc.vector.tensor_copy(ot[:], po[:])
                nc.sync.dma_start(OUT.ap()[t * 128:(t + 1) * 128, :], ot[:])
    nc.compile()
    return nc


_PERM = None


def _perm():
    """Within-head permutation: quadrant q holds pairs 16q..16q+15 as
    16 real rows then 16 imag rows."""
    global _PERM
    if _PERM is None:
        p = np.zeros(HD, dtype=np.int64)
        for q in range(4):
            for jj in range(16):
                p[32 * q + jj] = 2 * (16 * q + jj)
                p[32 * q + 16 + jj] = 2 * (16 * q + jj) + 1
        _PERM = p
    return _PERM


def make_inputs(x, freqs_cos, freqs_sin, wq, wk, wv, wo, q_norm_w, k_norm_w):
    perm = _perm()
    xT = np.ascontiguousarray(x.reshape(S, D).T)
    cosT = np.ascontiguousarray(freqs_cos.T)  # [64, S]
    sinT = np.ascontiguousarray(freqs_sin.T)
    # per-quadrant: rows 32q..32q+15 = cos[16q..16q+15], rows +16 same
    cc = np.empty((HD, S), dtype=np.float32)
    ssg = np.empty((HD, S), dtype=np.float32)
    for q in range(4):
        cc[32 * q:32 * q + 16] = cosT[16 * q:16 * q + 16]
        cc[32 * q + 16:32 * q + 32] = cosT[16 * q:16 * q + 16]
        ssg[32 * q:32 * q + 16] = -sinT[16 * q:16 * q + 16]
        ssg[32 * q + 16:32 * q + 32] = sinT[16 * q:16 * q + 16]
    # causal masks for diagonal chunks
    mask = np.empty((128, 4 * TB), dtype=np.float32)
    qt = np.arange(TB)
    for ci in range(4):
        kt = 128 * ci + np.arange(128)
        mask[:, ci * TB:(ci + 1) * TB] = np.where(
            kt[:, None] <= qt[None, :], 0.0, NEG).astype(np.float32)
    wqk = (q_norm_w * k_norm_w)[perm].reshape(1, HD).astype(np.float32)
    common = dict(
        XT=xT.astype(np.float32), CC=cc, SS=ssg, MASK=mask,
        IDM=np.eye(128, dtype=np.float32), WQKR=wqk,
        ONESR=np.ones((1, 128), dtype=np.float32),
        ONESC=np.ones((128, 1), dtype=np.float32),
    )
    in_maps = []
    for c in range(N_CORES):
        wq_c = wq[:, c * NH * HD:(c + 1) * NH * HD].reshape(D, NH, HD)
        wq_c = np.ascontiguousarray(wq_c[:, :, perm].reshape(D, NH * HD))
        wk_c = np.ascontiguousarray(wk[:, c * HD:(c + 1) * HD][:, perm])
        wv_c = np.ascontiguousarray(wv[:, c * HD:(c + 1) * HD])
        wo_c = np.ascontiguousarray(wo[:, c * 512:(c + 1) * 512])
        in_maps.append(dict(common, WQ=wq_c, WK=wk_c, WV=wv_c, WO=wo_c))
    return in_maps


_NC = None


def get_nc():
    global _NC
    if _NC is None:
        _NC = build_nc()
    return _NC


def kernel(x, freqs_cos, freqs_sin, input_pos, wq, wk, wv, wo,
           q_norm_w, k_norm_w, k_cache, v_cache):
    from concourse.bass_utils import run_bass_kernel_spmd
    nc = get_nc()
    in_maps = make_inputs(np.asarray(x), np.asarray(freqs_cos),
                          np.asarray(freqs_sin), np.asarray(wq),
                          np.asarray(wk), np.asarray(wv), np.asarray(wo),
                          np.asarray(q_norm_w), np.asarray(k_norm_w))
    res = run_bass_kernel_spmd(nc, in_maps, core_ids=list(range(N_CORES)))
    out = np.concatenate([res.results[c]["OUT"] for c in range(N_CORES)],
                         axis=1)
    return out.reshape(B, S, D).astype(np.float32)


# revision 8
# speedup vs baseline: 1.3171x; 1.3171x over previous
"""Trainium2 Bass kernel for nn_AttentionMHA: 8-way tensor-parallel over heads.

Full attention prefill: B=1, S=2048, D=4096, H=32 Q-heads, KVH=8 KV-heads,
HD=128, causal (input_pos = arange(S)).

Per-core sharding (core c of 8): Q heads 4c..4c+3, KV head c, wo columns
512c..512(c+1).  Software-pipelined per token block j:
  QKV pass1(j) -> QKV pass2(j) -> attention(j-1) -> AllGather(j-1)
  -> RoPE/RMSNorm processing(j) -> output-projection(j-2)
so the PE never waits on the cross-engine RoPE/softmax chains and the
per-block AllGathers overlap with compute.  ATTN_DT selects the datatype of
the large matmuls (bf16 = full PE rate, f32r = half rate but ~10x more
accurate); RoPE/softmax/norm arithmetic stays fp32.
"""
import os
import sys

sys.path.insert(0, "/opt/trn_rl_repo")

import numpy as np
import ml_dtypes

import concourse.bass as bass
import concourse.tile as tile
from concourse import bacc, mybir

f32 = mybir.dt.float32
f32r = mybir.dt.float32r
bf16 = mybir.dt.bfloat16
AF = mybir.ActivationFunctionType
ALU = mybir.AluOpType

B, S, D = 1, 2048, 4096
H, KVH, HD = 32, 8, 128
NH = 4            # q heads per core
TB = 512          # token block
NT = S // TB      # 4 token blocks
KC = D // 128     # 32 contraction chunks
NKT = S // 128    # 16 key chunks
EPS = 1e-5
SCALE = 1.0 / np.sqrt(HD)
NEG = -30000.0
N_CORES = 8

DT_BIG_NAME = os.environ.get("ATTN_DT", "bf16")

SWAP_MASK = list(range(16, 32)) + list(range(0, 16))


def build_nc(dt_name=None):
    dt_name = dt_name or DT_BIG_NAME
    dtb = bf16 if dt_name == "bf16" else f32r
    # DRAM dtype of the big inputs: bf16 tensors are host-converted; f32r is
    # a bitcast view of f32.
    dram_big = bf16 if dt_name == "bf16" else f32

    nc = bacc.Bacc("TRN2", target_bir_lowering=False, debug=False,
                   num_devices=N_CORES)

    XT = nc.dram_tensor("XT", [D, S], dram_big, kind="ExternalInput")
    WQ = nc.dram_tensor("WQ", [D, NH * HD], dram_big, kind="ExternalInput")
    WK = nc.dram_tensor("WK", [D, HD], dram_big, kind="ExternalInput")
    WV = nc.dram_tensor("WV", [D, HD], dram_big, kind="ExternalInput")
    WO = nc.dram_tensor("WO", [D, 512], dram_big, kind="ExternalInput")
    CC = nc.dram_tensor("CC", [HD, S], f32, kind="ExternalInput")
    SSI = nc.dram_tensor("SSI", [HD, S], f32, kind="ExternalInput")
    MASK = nc.dram_tensor("MASK", [128, 4 * TB], f32, kind="ExternalInput")
    IDM = nc.dram_tensor("IDM", [128, 128], f32, kind="ExternalInput")
    WQKR = nc.dram_tensor("WQKR", [1, 128], f32, kind="ExternalInput")
    ONESR = nc.dram_tensor("ONESR", [1, 128], f32, kind="ExternalInput")
    ONESC = nc.dram_tensor("ONESC", [128, 1], f32, kind="ExternalInput")
    OUT = nc.dram_tensor("OUT", [S, 512], f32, kind="ExternalOutput")

    def big_view(t):
        ap = t.ap()
        return ap if dtb is bf16 else ap.bitcast(f32r)

    with tile.TileContext(nc) as tc, \
         nc.allow_low_precision(reason="intentional bf16/f32r operand rounding"):
        from contextlib import ExitStack
        with tc.tile_pool(name="dram", bufs=1, space="DRAM") as dram:
            y_ag = [dram.tile([NH * HD, TB], dram_big, name=f"yag{j}")
                    for j in range(NT)]
            y_full = [dram.tile([H * HD, TB], dram_big, addr_space="Shared",
                                name=f"yfull{j}") for j in range(NT)]
            ctx = ExitStack()
            with ctx:
                const = ctx.enter_context(tc.tile_pool(name="const", bufs=1))
                wqpool = ctx.enter_context(tc.tile_pool(name="wqpool", bufs=1))
                wopool = ctx.enter_context(tc.tile_pool(name="wopool", bufs=1))
                xtp = ctx.enter_context(tc.tile_pool(name="xtp", bufs=6))
                wkvp = ctx.enter_context(tc.tile_pool(name="wkvp", bufs=4))
                qfp = ctx.enter_context(tc.tile_pool(name="qfp", bufs=8))
                resid = ctx.enter_context(tc.tile_pool(name="resid", bufs=1))
                tmp = ctx.enter_context(tc.tile_pool(name="tmp", bufs=2))
                smalls = ctx.enter_context(tc.tile_pool(name="smalls", bufs=2))
                expp = ctx.enter_context(tc.tile_pool(name="expp", bufs=4))
                ystp = ctx.enter_context(tc.tile_pool(name="ystp", bufs=2))
                ytp = ctx.enter_context(tc.tile_pool(name="ytp", bufs=2))
                outp = ctx.enter_context(tc.tile_pool(name="outp", bufs=2))
                # PSUM: proj(3) + scores(2) + y(1) + bcast(1) + dp/po(1) = 8
                proj = ctx.enter_context(
                    tc.tile_pool(name="proj", bufs=1, space="PSUM"))
                scoresp = ctx.enter_context(
                    tc.tile_pool(name="scoresp", bufs=2, space="PSUM"))
                ypp = ctx.enter_context(
                    tc.tile_pool(name="ypp", bufs=1, space="PSUM"))
                bcp = ctx.enter_context(
                    tc.tile_pool(name="bcp", bufs=1, space="PSUM"))
                dpp = ctx.enter_context(
                    tc.tile_pool(name="dpp", bufs=1, space="PSUM"))

                # ---- constants ----
                cc_t = const.tile([HD, S], f32)
                nc.sync.dma_start(cc_t[:], CC.ap())
                ss_t = const.tile([HD, S], f32)
                nc.sync.dma_start(ss_t[:], SSI.ap())
                mask_t = const.tile([128, 4 * TB], f32)
                nc.sync.dma_start(mask_t[:], MASK.ap())
                id_t = const.tile([128, 128], f32)
                nc.sync.dma_start(id_t[:], IDM.ap())
                wqk_t = const.tile([1, 128], f32r)
                nc.sync.dma_start(wqk_t[:], WQKR.ap().bitcast(f32r))
                onesr_t = const.tile([1, 128], f32r)
                nc.sync.dma_start(onesr_t[:], ONESR.ap().bitcast(f32r))
                onesc_t = const.tile([128, 1], f32r)
                nc.sync.dma_start(onesc_t[:], ONESC.ap().bitcast(f32r))
                onesc_b = const.tile([128, 1], dtb)
                nc.vector.tensor_copy(onesc_b[:], onesc_t[:].bitcast(f32))
                eps_t = const.tile([1, 1], f32)
                nc.vector.memset(eps_t[:], EPS)

                wq_t = wqpool.tile([128, KC, NH * HD], dtb)
                nc.sync.dma_start(
                    wq_t[:], big_view(WQ).rearrange("(k p) n -> p k n", p=128))
                wo_t = wopool.tile([128, KC, 512], dtb)
                nc.sync.dma_start(
                    wo_t[:], big_view(WO).rearrange("(k p) n -> p k n", p=128))

                kfin = resid.tile([128, S], dtb)
                vnat = resid.tile([128, NKT * 128], dtb)

                xt_src = big_view(XT).rearrange("(k p) t -> k p t", p=128)
                wk_src = big_view(WK).rearrange("(k p) n -> k p n", p=128)
                wv_src = big_view(WV).rearrange("(k p) n -> k p n", p=128)

                def process_qk(raw_psum, is_k, j):
                    """RoPE + RMSNorm from raw projection psum [128, TB]."""
                    qs = tmp.tile([128, TB], f32, tag="qs")
                    nc.scalar.copy(qs[:], raw_psum[:])
                    sq = tmp.tile([128, TB], f32r, tag="sq")
                    nc.scalar.square(sq[:], raw_psum[:])
                    rsp = bcp.tile([1, TB], f32, tag="bc")
                    nc.tensor.matmul(rsp[:], onesc_t[:], sq[:],
                                     start=True, stop=True)
                    srt = smalls.tile([1, TB], f32, tag="srt")
                    nc.scalar.activation(srt[:], rsp[:], AF.Sqrt,
                                         bias=eps_t[:], scale=1.0 / HD)
                    recr = smalls.tile([1, TB], f32r, tag="recr")
                    nc.vector.reciprocal(recr[:], srt[:])
                    bc = bcp.tile([128, TB], f32, tag="bc")
                    lhs = wqk_t if is_k else onesr_t
                    nc.tensor.matmul(bc[:], lhs[:], recr[:],
                                     start=True, stop=True)
                    tsw = tmp.tile([128, TB], f32, tag="tsw")
                    nc.vector.stream_shuffle(tsw[:], qs[:], SWAP_MASK)
                    t1 = tmp.tile([128, TB], f32, tag="t1")
                    nc.vector.tensor_tensor(
                        t1[:], qs[:], cc_t[:, j * TB:(j + 1) * TB], ALU.mult)
                    t2 = tmp.tile([128, TB], f32, tag="t2")
                    nc.vector.tensor_tensor(
                        t2[:], tsw[:], ss_t[:, j * TB:(j + 1) * TB], ALU.mult)
                    nc.vector.tensor_tensor(t1[:], t1[:], t2[:], ALU.add)
                    if is_k:
                        nc.vector.tensor_tensor(
                            kfin[:, j * TB:(j + 1) * TB], t1[:], bc[:],
                            ALU.mult)
                        return None
                    qf = qfp.tile([128, TB], dtb, tag="qf")
                    nc.vector.tensor_tensor(qf[:], t1[:], bc[:], ALU.mult)
                    return qf

                def emit_qkv(j):
                    tok = slice(j * TB, (j + 1) * TB)
                    ps = []
                    for pidx in range(2):
                        pa = proj.tile([128, TB], f32, tag="pa")
                        pb = proj.tile([128, TB], f32, tag="pb")
                        pc = proj.tile([128, TB], f32, tag="pc")
                        ps.append((pa, pb, pc))
                        h0 = 2 * pidx
                        for k in range(KC):
                            xt = xtp.tile([128, TB], dtb, tag="xt")
                            nc.sync.dma_start(xt[:], xt_src[k][:, tok])
                            wkv = wkvp.tile([128, HD], dtb, tag="wkv")
                            nc.sync.dma_start(
                                wkv[:], wk_src[k] if pidx == 0 else wv_src[k])
                            st, sp = (k == 0), (k == KC - 1)
                            nc.tensor.matmul(
                                pa[:], wq_t[:, k, h0 * 128:(h0 + 1) * 128],
                                xt[:], start=st, stop=sp)
                            nc.tensor.matmul(
                                pb[:],
                                wq_t[:, k, (h0 + 1) * 128:(h0 + 2) * 128],
                                xt[:], start=st, stop=sp)
                            nc.tensor.matmul(pc[:], wkv[:], xt[:],
                                             start=st, stop=sp)
                    return ps

                def emit_proc(j, ps):
                    (pq0, pq1, pk), (pq2, pq3, pv) = ps
                    process_qk(pk, True, j)
                    q_tiles = [process_qk(pq0, False, j),
                               process_qk(pq1, False, j),
                               process_qk(pq2, False, j),
                               process_qk(pq3, False, j)]
                    vt_s = tmp.tile([128, TB], f32, tag="vts")
                    nc.scalar.copy(vt_s[:], pv[:])
                    for ci in range(4):
                        pt = bcp.tile([128, 128], f32, tag="bc")
                        nc.tensor.transpose(
                            pt[:], vt_s[:, ci * 128:(ci + 1) * 128], id_t[:])
                        nc.vector.tensor_copy(
                            vnat[:, (4 * j + ci) * 128:(4 * j + ci + 1) * 128],
                            pt[:])
                    return q_tiles

                def emit_attention(j, q_tiles):
                    nchunks = 4 * (j + 1)
                    for h in range(NH):
                        qf = q_tiles[h]
                        yp = ypp.tile([128, TB], f32, tag="yp")
                        dps = dpp.tile([1, TB], f32, tag="dp")
                        for c in range(nchunks):
                            sc = scoresp.tile([128, TB], f32, tag="sc")
                            nc.tensor.matmul(
                                sc[:], kfin[:, c * 128:(c + 1) * 128], qf[:],
                                start=True, stop=True)
                            if c >= 4 * j:
                                ci = c - 4 * j
                                nc.vector.tensor_tensor(
                                    sc[:], sc[:],
                                    mask_t[:, ci * TB:(ci + 1) * TB], ALU.add)
                            ex = expp.tile([128, TB], dtb, tag="ex")
                            nc.scalar.activation(ex[:], sc[:], AF.Exp,
                                                 scale=SCALE)
                            nc.tensor.matmul(
                                yp[:], vnat[:, c * 128:(c + 1) * 128], ex[:],
                                start=(c == 0), stop=(c == nchunks - 1))
                            nc.tensor.matmul(
                                dps[:], onesc_b[:], ex[:],
                                start=(c == 0), stop=(c == nchunks - 1))
                        drecr = smalls.tile([1, TB], f32r, tag="drecr")
                        nc.vector.reciprocal(drecr[:], dps[:])
                        bcr = bcp.tile([128, TB], f32, tag="bc")
                        nc.tensor.matmul(bcr[:], onesr_t[:], drecr[:],
                                         start=True, stop=True)
                        bcs = tmp.tile([128, TB], f32, tag="bcs")
                        nc.scalar.copy(bcs[:], bcr[:])
                        yst = ystp.tile([128, TB], dram_big, tag="yst")
                        nc.vector.tensor_tensor(yst[:], yp[:], bcs[:],
                                                ALU.mult)
                        nc.sync.dma_start(y_ag[j][h * HD:(h + 1) * HD, :],
                                          yst[:])
                    nc.gpsimd.collective_compute(
                        "AllGather", ALU.bypass,
                        replica_groups=[list(range(N_CORES))],
                        ins=[y_ag[j][:]], outs=[y_full[j][:]])

                def emit_wo(j):
                    src = y_full[j][:]
                    if dtb is f32r:
                        src = src.bitcast(f32r)
                    src = src.rearrange("(k p) t -> p k t", p=128)
                    for ti in range(4):
                        yt = ytp.tile([128, KC, 128], dtb, tag="yt")
                        nc.sync.dma_start(
                            yt[:], src[:, :, ti * 128:(ti + 1) * 128])
                        po = dpp.tile([128, 512], f32, tag="dp")
                        for k in range(KC):
                            nc.tensor.matmul(po[:], yt[:, k, :], wo_t[:, k, :],
                                             start=(k == 0), stop=(k == KC - 1))
                        ot = outp.tile([128, 512], f32, tag="ot")
                        nc.vector.tensor_copy(ot[:], po[:])
                        t = 4 * j + ti
                        nc.sync.dma_start(OUT.ap()[t * 128:(t + 1) * 128, :],
                                          ot[:])

                # ---- software-pipelined emission ----
                prev_q = None
                for j in range(NT):
                    ps = emit_qkv(j)
                    if j >= 1:
                        emit_attention(j - 1, prev_q)
                    prev_q = emit_proc(j, ps)
                    if j >= 2:
                        emit_wo(j - 2)
                emit_attention(NT - 1, prev_q)
                emit_wo(NT - 2)
                emit_wo(NT - 1)

    nc.compile()
    return nc


_PERM = None


def _perm():
    """Within-head permutation: quadrant q holds pairs 16q..16q+15 as
    16 real rows then 16 imag rows (stream_shuffle swaps within quadrants)."""
    global _PERM
    if _PERM is None:
        p = np.zeros(HD, dtype=np.int64)
        for q in range(4):
            for jj in range(16):
                p[32 * q + jj] = 2 * (16 * q + jj)
                p[32 * q + 16 + jj] = 2 * (16 * q + jj) + 1
        _PERM = p
    return _PERM


def make_inputs(x, freqs_cos, freqs_sin, wq, wk, wv, wo, q_norm_w, k_norm_w,
                dt_name=None):
    dt_name = dt_name or DT_BIG_NAME
    np_big = ml_dtypes.bfloat16 if dt_name == "bf16" else np.float32
    perm = _perm()
    xT = np.ascontiguousarray(x.reshape(S, D).T).astype(np_big)
    cosT = np.ascontiguousarray(freqs_cos.T)  # [64, S]
    sinT = np.ascontiguousarray(freqs_sin.T)
    cc = np.empty((HD, S), dtype=np.float32)
    ssg = np.empty((HD, S), dtype=np.float32)
    for q in range(4):
        cc[32 * q:32 * q + 16] = cosT[16 * q:16 * q + 16]
        cc[32 * q + 16:32 * q + 32] = cosT[16 * q:16 * q + 16]
        ssg[32 * q:32 * q + 16] = -sinT[16 * q:16 * q + 16]
        ssg[32 * q + 16:32 * q + 32] = sinT[16 * q:16 * q + 16]
    mask = np.empty((128, 4 * TB), dtype=np.float32)
    qt = np.arange(TB)
    for ci in range(4):
        kt = 128 * ci + np.arange(128)
        mask[:, ci * TB:(ci + 1) * TB] = np.where(
            kt[:, None] <= qt[None, :], 0.0, NEG).astype(np.float32)
    wqk = (q_norm_w * k_norm_w)[perm].reshape(1, HD).astype(np.float32)
    common = dict(
        XT=xT, CC=cc, SSI=ssg, MASK=mask,
        IDM=np.eye(128, dtype=np.float32), WQKR=wqk,
        ONESR=np.ones((1, 128), dtype=np.float32),
        ONESC=np.ones((128, 1), dtype=np.float32),
    )
    in_maps = []
    for c in range(N_CORES):
        wq_c = wq[:, c * NH * HD:(c + 1) * NH * HD].reshape(D, NH, HD)
        wq_c = np.ascontiguousarray(wq_c[:, :, perm].reshape(D, NH * HD))
        wk_c = np.ascontiguousarray(wk[:, c * HD:(c + 1) * HD][:, perm])
        wv_c = np.ascontiguousarray(wv[:, c * HD:(c + 1) * HD])
        wo_c = np.ascontiguousarray(wo[:, c * 512:(c + 1) * 512])
        in_maps.append(dict(
            common, WQ=wq_c.astype(np_big), WK=wk_c.astype(np_big),
            WV=wv_c.astype(np_big), WO=wo_c.astype(np_big)))
    return in_maps


_NC = None


def get_nc():
    global _NC
    if _NC is None:
        _NC = build_nc()
    return _NC


def kernel(x, freqs_cos, freqs_sin, input_pos, wq, wk, wv, wo,
           q_norm_w, k_norm_w, k_cache, v_cache):
    from concourse.bass_utils import run_bass_kernel_spmd
    nc = get_nc()
    in_maps = make_inputs(np.asarray(x), np.asarray(freqs_cos),
                          np.asarray(freqs_sin), np.asarray(wq),
                          np.asarray(wk), np.asarray(wv), np.asarray(wo),
                          np.asarray(q_norm_w), np.asarray(k_norm_w))
    res = run_bass_kernel_spmd(nc, in_maps, core_ids=list(range(N_CORES)))
    out = np.concatenate([res.results[c]["OUT"] for c in range(N_CORES)],
                         axis=1)
    return out.reshape(B, S, D).astype(np.float32)
